# revision 1
# baseline (speedup 1.0000x reference)
"""24x24-bit array multiplier on 8 TRN2 NeuronCores (Bass).

A, B: [65536, 24] float32 0/1 bit-vectors (LSB first) -> P: [65536, 48] bits.

Pure data parallel: each core gets 8192 rows laid out as
[128 partitions x 64 rows/partition].  Three decoupled, pipelined
granularities per core:

  LOADS:  few big A/B DMA pairs (HWDGE issue cadence ~650ns makes many
          small loads slow; DMA device is the serialized bottleneck).
  SCAN GROUPS (sub-ranges of a load pair), per group on DVE:
    scan   tensor_tensor_scan per input: bits -> two 12-bit limbs
           (scaled 2^-11), one op per input
    prod   STT (la*2^22)*lb: all 4 limb cross products -> int32
    sh     TSP >>12 high digits -> mega tile sh32 [P,4,64]
    q      STT mod-4096 low digits -> int16 mega G [P,14,64]
    carry  8 small ops, radix-4096 digit-serial combine (the band only
           reads bits 0..11, so raw carry sums serve as digits)
  OUT SETS (sub-ranges of finished scan groups):
    band   ONE int16 TT AND vs pre-expanded mask (2x DVE mode or Pool)
    sign   Act Sign int16 -> fp32 0/1, transposing out AP (or DVE !=0)
    store  contiguous rows-per-partition DMA
"""

import numpy as np

import concourse.bacc as bacc
import concourse.bass as bass
import concourse.mybir as mybir
import concourse.tile as tile
from concourse.alu_op_type import AluOpType
from concourse.bass_utils import run_bass_kernel_spmd

P = 128           # SBUF partitions
C = 64            # batch rows per partition
NB = P * C        # rows per core = 8192
N_CORES = 8
BATCH = NB * N_CORES

f32 = mybir.dt.float32
i32 = mybir.dt.int32
i16 = mybir.dt.int16

AND = AluOpType.bitwise_and
SHR = AluOpType.logical_shift_right
SHL = AluOpType.logical_shift_left
ADD, MUL = AluOpType.add, AluOpType.mult
NE = AluOpType.not_equal
SIGN = mybir.ActivationFunctionType.Sign

DEFAULT_BUILD = dict(
    load_sizes=[24, 32, 8],
    scan_groups=[(0, 12, 0), (12, 12, 0), (24, 16, 1), (40, 16, 1),
                 (56, 8, 2)],
    carry_sizes=[24, 32, 8],
    carry_eng=["vector"] * 3,
    out_sizes=[12, 12, 10, 10, 12, 8],
    band_eng=["vector"] * 6,
    sign_eng=["scalar"] * 6,
    out_eng=["sync"] * 6,
)


def build_nc(load_sizes=None, scan_groups=None, out_sizes=None,
             carry_sizes=None, carry_eng=None,
             band_eng=None, sign_eng=None, out_eng=None,
             scan_combined: bool = False,
             group_order=None) -> bass.Bass:
    """One SPMD program; every core runs it on its own shard."""
    nc = bacc.Bacc(
        "TRN2",
        target_bir_lowering=False,
        debug=False,
        num_devices=N_CORES,
    )
    A = nc.declare_dram_parameter("A", [NB, 24], f32, isOutput=False)
    B = nc.declare_dram_parameter("B", [NB, 24], f32, isOutput=False)
    OUT = nc.declare_dram_parameter("out", [NB, 48], f32, isOutput=True)

    d = DEFAULT_BUILD
    load_sizes = load_sizes or d["load_sizes"]
    scan_groups = scan_groups or d["scan_groups"]
    out_sizes = out_sizes or d["out_sizes"]
    carry_sizes = carry_sizes or d["carry_sizes"]
    n_s = len(out_sizes)
    carry_eng = carry_eng or ["vector"] * len(carry_sizes)
    band_eng = band_eng or ["vector"] * n_s
    sign_eng = sign_eng or ["scalar"] * n_s
    out_eng = out_eng or ["sync"] * n_s
    assert sum(load_sizes) == C and sum(out_sizes) == C
    assert sum(g[1] for g in scan_groups) == C
    assert sum(carry_sizes) == C
    coff = [0]
    for x in carry_sizes:
        coff.append(coff[-1] + x)
    ccL = max(load_sizes)
    ccG = max(g[1] for g in scan_groups)
    ccO = max(max(out_sizes), max(carry_sizes))

    loff = [0]
    for x in load_sizes:
        loff.append(loff[-1] + x)
    soff = [0]
    for x in out_sizes:
        soff.append(soff[-1] + x)
    # scan group sanity: inside its load chunk, in row order
    r = 0
    for (g0, gc, li) in scan_groups:
        assert g0 == r, (g0, r)
        assert loff[li] <= g0 and g0 + gc <= loff[li + 1]
        r += gc

    Av = A[:].rearrange("(p c) b -> p c b", p=P)
    Bv = B[:].rearrange("(p c) b -> p c b", p=P)
    Ov = OUT[:].rearrange("(p c) b -> p c b", p=P)

    with tile.TileContext(nc) as tc, \
            tc.tile_pool(name="const", bufs=1) as cpool, \
            tc.tile_pool(name="work", bufs=1) as pool:
        iot = cpool.tile([P, 12], i32)
        ones = cpool.tile([P, 12], i32)
        mask12 = cpool.tile([P, 12], i32)
        mask_full = cpool.tile([P, 12 * ccO], i16)
        sc0 = cpool.tile([P, 2 * ccG * 24], f32)
        c12 = cpool.tile([P, C], i16)      # shift-amount 12 for Pool SHR

        def emit_consts():
            # scan multiplier stream: 0.5 everywhere, 0 at 12-bit segment
            # starts (resets the recurrence per limb)
            nc.gpsimd.memset(c12[:], 12)
            nc.gpsimd.memset(sc0[:], 0.5)
            sc0v = sc0[:].rearrange("p (c l b) -> p c l b", l=2, b=12)
            nc.gpsimd.memset(sc0v[:, :, :, 0], 0.0)
            # mask_full[p, k, c] = 1 << k, expanded over c (so the band's
            # innermost axis is packed -> 2x DVE mode / single Pool op)
            nc.gpsimd.iota(iot[:], [[1, 12]], channel_multiplier=0)
            nc.vector.memset(ones[:], 1)
            nc.vector.tensor_tensor(mask12[:], ones[:], iot[:], SHL)
            m12b = mask12[:].rearrange("p k -> p k ()").broadcast_to(
                (P, 12, ccO))
            nc.gpsimd.tensor_copy(
                mask_full[:].rearrange("p (k c) -> p k c", c=ccO), m12b)

        # mega tiles shared by all groups (disjoint column ranges)
        sh32 = pool.tile([P, 4 * C], i32)
        sh32v = sh32[:].rearrange("p (k c) -> p k c", k=4)
        G = pool.tile([P, 14 * C], i16)
        Gv = G[:].rearrange("p (s c) -> p s c", s=14)
        # G slots: 0..3 digits (d0=q0,d1,d2,d3), 4/8/12 q1/q2/q3,
        # 7 t1a, 9 u2a, 10 u2, 11 c1, 13 c2

        ab_t = {}

        def s_load(li):
            cc = load_sizes[li]
            rows = slice(loff[li], loff[li + 1])
            ab = pool.tile([P, 2 * cc * 24], f32, tag=f"ab{li}")
            ab_t[li] = ab[:].rearrange("p (i c b) -> p i c b", i=2, b=24)
            nc.sync.dma_start(ab_t[li][:, 0], Av[:, rows])
            nc.sync.dma_start(ab_t[li][:, 1], Bv[:, rows])

        lm = {}

        def s_scan(g, half):
            g0, gc, li = scan_groups[g]
            if g not in lm:
                so = pool.tile([P, 2 * gc * 24], f32, tag=f"so{g}")
                lm[g] = so
            so = lm[g]
            off = g0 - loff[li]
            if half is None:   # combined A+B in one scan op
                src = ab_t[li][:, :, off:off + gc].rearrange(
                    "p i c b -> p (i c b)")
                nc.vector.tensor_tensor_scan(
                    so[:], sc0[:][:, :2 * gc * 24], src, 0.0, MUL, ADD)
                return
            src = ab_t[li][:, half, off:off + gc].rearrange(
                "p c b -> p (c b)")
            nc.vector.tensor_tensor_scan(
                so[:][:, half * gc * 24:(half + 1) * gc * 24],
                sc0[:][:, :gc * 24], src, 0.0, MUL, ADD)

        def s_prodshq(g):
            g0, gc, li = scan_groups[g]
            r0, r1 = g0, g0 + gc
            so4 = lm[g][:].rearrange("p (i c l b) -> p i c l b",
                                     i=2, l=2, b=12)
            lmA = so4[:, 0, :, :, 11]     # [P, gc, 2] limbs * 2^-11
            lmB = so4[:, 1, :, :, 11]
            pt = pool.tile([P, gc * 4], i32, tag=f"pt{g}")
            pt4 = pt[:].rearrange("p (c i l) -> p c i l", i=2, l=2)
            # (la*2^22)*lb == limb_a * limb_b exactly (la*2^22 = a*2^11<2^23)
            # STT is 3D-max on walrus: one op per a-limb
            for i in range(2):
                la_i = lmA[:, :, i:i + 1].broadcast_to((P, gc, 2))
                nc.vector.scalar_tensor_tensor(pt4[:, :, i], la_i,
                                               float(2 ** 22), lmB, MUL, MUL)
            ptT = pt[:].rearrange("p (c k) -> p k c", k=4)
            shg = sh32v[:, :, r0:r1]
            nc.vector.tensor_scalar(shg, ptT, 12, None, SHR)
            # q -> G slots (0,4,8,12): digit0 plus q1..q3
            nc.vector.scalar_tensor_tensor(Gv[:, 0:13:4, r0:r1], shg,
                                           -4096.0, ptT, MUL, ADD)

        def s_carry(j):
            cols = slice(coff[j], coff[j + 1])

            def gq(s):
                return Gv[:, s:s + 1, cols]

            def sv(k):
                return sh32v[:, k:k + 1, cols]

            v_ = getattr(nc, carry_eng[j])

            def shr12(dst, src):
                if carry_eng[j] == "vector":
                    nc.vector.tensor_scalar(dst, src, 12, None, SHR)
                else:
                    v_.tensor_tensor(dst, src,
                                     c12[:][:, cols].rearrange("p c -> p () c"),
                                     SHR)

            v_.tensor_tensor(gq(7), gq(4), gq(8), ADD)      # t1a = q1+q2
            v_.tensor_tensor(gq(9), sv(1), sv(2), ADD)      # u2a = sh1+sh2
            v_.tensor_tensor(gq(1), gq(7), sv(0), ADD)      # d1 = t1a+sh0
            v_.tensor_tensor(gq(10), gq(9), gq(12), ADD)    # u2 = u2a+q3
            shr12(gq(11), gq(1))                            # c1
            v_.tensor_tensor(gq(2), gq(10), gq(11), ADD)    # d2 = u2+c1
            shr12(gq(13), gq(2))                            # c2
            v_.tensor_tensor(gq(3), sv(3), gq(13), ADD)     # d3 = sh3+c2

        bt_t = {}

        def s_band(s):
            oc = out_sizes[s]
            o0, o1 = soff[s], soff[s + 1]
            bt = pool.tile([P, 4 * 12 * oc], i16, tag=f"bt{s}")
            bt4 = bt[:].rearrange("p (l k c) -> p l k c", l=4, k=12)
            bt_t[s] = bt4
            digits = Gv[:, 0:4, o0:o1].rearrange("p l c -> p l () c")
            maskv = mask_full[:].rearrange(
                "p (k c) -> p () k c", c=ccO)[:, :, :, :oc]
            getattr(nc, band_eng[s]).tensor_tensor(
                bt4, digits.broadcast_to((P, 4, 12, oc)),
                maskv.broadcast_to((P, 4, 12, oc)), AND)

        ob_t = {}

        def s_sign(s):
            oc = out_sizes[s]
            ob = pool.tile([P, oc * 48], f32, tag=f"ob{s}")
            ob_t[s] = ob
            obT = ob[:].rearrange("p (c l k) -> p l k c", l=4, k=12)
            if sign_eng[s] == "scalar":
                nc.scalar.activation(obT, bt_t[s], SIGN)
            else:
                nc.vector.tensor_scalar(obT, bt_t[s], 0, None, NE)

        def s_out(s):
            o0, o1 = soff[s], soff[s + 1]
            getattr(nc, out_eng[s]).dma_start(
                Ov[:, o0:o1],
                ob_t[s][:].rearrange("p (c b) -> p c b", b=48))

        # ---- emission ----
        for li in range(len(load_sizes)):
            s_load(li)
        emit_consts()

        covered = set()
        carried = set()
        carry_emitted = [False] * len(carry_sizes)
        bs_emitted = [False] * n_s
        store_next = 0
        order = group_order or list(range(len(scan_groups)))
        for g in order:
            if scan_combined:
                s_scan(g, None)
            else:
                s_scan(g, 0)
                s_scan(g, 1)
            s_prodshq(g)
            g0, gc, _ = scan_groups[g]
            covered |= set(range(g0, g0 + gc))
            for j in range(len(carry_sizes)):
                if not carry_emitted[j] and \
                        set(range(coff[j], coff[j + 1])) <= covered:
                    s_carry(j)
                    carry_emitted[j] = True
                    carried |= set(range(coff[j], coff[j + 1]))
            for s in range(n_s):
                if not bs_emitted[s] and \
                        set(range(soff[s], soff[s + 1])) <= carried:
                    s_band(s)
                    s_sign(s)
                    bs_emitted[s] = True
            while store_next < n_s and bs_emitted[store_next]:
                s_out(store_next)
                store_next += 1

    nc.compile()
    return nc


_CACHE: dict = {}


def kernel(A: np.ndarray, B: np.ndarray) -> np.ndarray:
    A = np.ascontiguousarray(np.asarray(A, dtype=np.float32))
    B = np.ascontiguousarray(np.asarray(B, dtype=np.float32))
    assert A.shape == (BATCH, 24) and B.shape == (BATCH, 24), (A.shape, B.shape)

    if "nc" not in _CACHE:
        _CACHE["nc"] = build_nc(**DEFAULT_BUILD)
    nc = _CACHE["nc"]

    in_maps = []
    for i in range(N_CORES):
        sl = slice(i * NB, (i + 1) * NB)
        in_maps.append({"A": A[sl], "B": B[sl]})

    res = run_bass_kernel_spmd(nc, in_maps, core_ids=list(range(N_CORES)))
    outs = [np.asarray(res.results[i]["out"]) for i in range(N_CORES)]
    return np.concatenate(outs, axis=0).astype(np.float32)


if __name__ == "__main__":
    rng = np.random.default_rng(0)
    A = rng.integers(0, 2, (BATCH, 24)).astype(np.float32)
    B = rng.integers(0, 2, (BATCH, 24)).astype(np.float32)
    out = kernel(A, B)
    pw = (1 << np.arange(24)).astype(np.int64)
    a = (A.astype(np.int64) * pw).sum(-1)
    b = (B.astype(np.int64) * pw).sum(-1)
    p = a * b
    exp = ((p[:, None] >> np.arange(48)[None, :]) & 1).astype(np.float32)
    print("max abs diff:", np.abs(out - exp).max())
    assert np.array_equal(out, exp), "MISMATCH"
    print("EXACT MATCH")



# revision 2
# speedup vs baseline: 1.0319x; 1.0319x over previous
"""24x24-bit array multiplier on 8 TRN2 NeuronCores (Bass) — v2.

A, B: [65536, 24] float32 0/1 bit-vectors (LSB first) -> P: [65536, 48] bits.

Pure data parallel: each core gets 8192 rows as [128 part x 64 rows].
Math (exact): per row a = a0 + a1*2^12 (limbs via tensor_tensor_scan,
scaled 2^-11); 4 limb cross-products in f32 (exact, < 2^24) ->
radix-4096 digit lattice in i16 (every integer sum < 2^15: the DVE int
ADD runs through the fp32 datapath, so sums >= 2^24 would round) ->
per-digit 12-bit AND-mask expand (i16, 2x DVE mode) -> Sign (Act) or
not_equal (DVE) -> f32 bits.

Engine legality (neuronxcc): scans/shifts/bitwise/STT are DVE-only;
Act only does activation(func(scale*x+bias)); Pool does memset/iota/
converting-copies/f32-or-i32-matching arith TT and SWDGE DMA issue.

Schedule: loads tapered; first-chunk B load issued via Pool SWDGE so
both first transfers land early; all stage boundaries and queue
assignments are knobs; emission optionally zips the serial carry
lattice with small front/out ops so the static scheduler cannot
stretch it with 1.3us scan ops.
"""

import numpy as np

import concourse.bacc as bacc
import concourse.bass as bass
import concourse.mybir as mybir
import concourse.tile as tile
from concourse.alu_op_type import AluOpType
from concourse.bass_utils import run_bass_kernel_spmd

P = 128           # SBUF partitions
C = 64            # batch rows per partition
NB = P * C        # rows per core = 8192
N_CORES = 8
BATCH = NB * N_CORES

f32 = mybir.dt.float32
i32 = mybir.dt.int32
i16 = mybir.dt.int16

AND = AluOpType.bitwise_and
SHR = AluOpType.logical_shift_right
SHL = AluOpType.logical_shift_left
ADD, MUL = AluOpType.add, AluOpType.mult
NE = AluOpType.not_equal
SIGN = mybir.ActivationFunctionType.Sign

LABELS: dict = {}


def L(inst, label):
    try:
        LABELS[inst.ins.name] = label
    except AttributeError:
        pass
    return inst


DEFAULT_BUILD = dict(
    zip_emission=True,
    # (rows, engine_A, engine_B); engines in {sync, scalar, gpsimd}
    load_chunks=[(8, "sync", "gpsimd"), (24, "sync", "sync"),
                 (24, "sync", "sync"), (8, "sync", "sync")],
    # (rows, mode): aligned 1:1 with load_chunks; "split" = 2 scan ops
    # (A scan can start before B lands), "comb" = 1 op over [A|B]
    scan_chunks=[(8, "split", "-"), (24, "split", "-"),
                 (24, "split", "-"), (8, "split", "-")],
    # rows per front batch (prod/sh/q; must nest in a load chunk)
    mid_batches=[(8, "vector", "vector"), (24, "vector", "vector"),
                 (24, "vector", "vector"), (8, "vector", "vector")],
    # rows per carry batch (boundaries must be reachable from fronts)
    carry_batches=[32, 24, 8],
    # (rows, band_eng, sign_eng, store_eng); sign in {scalar, vector}
    out_sets=[(8, "vector", "scalar", "sync"),
              (12, "vector", "scalar", "sync"),
              (12, "vector", "scalar", "sync"),
              (12, "vector", "scalar", "sync"),
              (12, "vector", "scalar", "sync"),
              (8, "vector", "vector", "sync")],
)


def _offsets(entries):
    off = [0]
    for x in entries:
        off.append(off[-1] + (x[0] if isinstance(x, tuple) else x))
    return off


def build_nc(load_chunks=None, scan_chunks=None, mid_batches=None,
             out_sets=None, carry_batches=None,
             zip_emission=None) -> bass.Bass:
    """One SPMD program; every core runs it on its own shard."""
    d = DEFAULT_BUILD
    if zip_emission is None:
        zip_emission = d.get("zip_emission", False)
    load_chunks = load_chunks or d["load_chunks"]
    scan_chunks = scan_chunks or d["scan_chunks"]
    mid_batches = mid_batches or d["mid_batches"]
    out_sets = out_sets or d["out_sets"]
    carry_batches = carry_batches or d["carry_batches"]

    loff = _offsets(load_chunks)
    soff = _offsets(scan_chunks)
    moff = _offsets(mid_batches)
    ooff = _offsets(out_sets)
    coff = _offsets(carry_batches)
    assert loff[-1] == C and soff[-1] == C
    assert moff[-1] == C and ooff[-1] == C and coff[-1] == C
    # scan chunks align 1:1 with load chunks; mid batches nest in them
    assert [x[0] for x in scan_chunks] == [x[0] for x in load_chunks]
    for m in range(len(mid_batches)):
        li = max(i for i in range(len(load_chunks)) if loff[i] <= moff[m])
        assert moff[m + 1] <= loff[li + 1], (m, li, moff, loff)
    ccO = max(x[0] for x in out_sets)

    nc = bacc.Bacc(
        "TRN2",
        target_bir_lowering=False,
        debug=False,
        num_devices=N_CORES,
    )
    A = nc.declare_dram_parameter("A", [NB, 24], f32, isOutput=False)
    B = nc.declare_dram_parameter("B", [NB, 24], f32, isOutput=False)
    OUT = nc.declare_dram_parameter("out", [NB, 48], f32, isOutput=True)

    Av = A[:].rearrange("(p c) b -> p c b", p=P)
    Bv = B[:].rearrange("(p c) b -> p c b", p=P)
    Ov = OUT[:].rearrange("(p c) b -> p c b", p=P)

    with tile.TileContext(nc) as tc, \
            tc.tile_pool(name="const", bufs=1) as cpool, \
            tc.tile_pool(name="work", bufs=1) as pool:
        # ---------- load DMAs first (issue ASAP) ----------
        ab_t = {}
        for li, (cc, engA, engB) in enumerate(load_chunks):
            rows = slice(loff[li], loff[li + 1])
            ab = pool.tile([P, 2 * cc * 24], f32, tag=f"ab{li}")
            ab_t[li] = ab[:].rearrange("p (i c b) -> p i c b", i=2, b=24)
            L(getattr(nc, engA).dma_start(ab_t[li][:, 0], Av[:, rows]),
              f"loadA{li}")
            L(getattr(nc, engB).dma_start(ab_t[li][:, 1], Bv[:, rows]),
              f"loadB{li}")

        # ---------- constants ----------
        iot = cpool.tile([P, 12], i32)
        ones = cpool.tile([P, 12], i32)
        mask12 = cpool.tile([P, 12], i32)
        mask_full = cpool.tile([P, 12 * ccO], i16)
        scmax = max((2 if x[1] == "comb" else 1) * x[0]
                    for x in scan_chunks) * 24
        sc0 = cpool.tile([P, scmax], f32)

        # scan multiplier stream on Pool (DVE is the bottleneck engine):
        # 0.5 everywhere, 0.0 at each 12-bit limb start (resets recurrence)
        nc.gpsimd.memset(sc0[:], 0.5)
        sc0v = sc0[:].rearrange("p (c l b) -> p c l b", l=2, b=12)
        nc.gpsimd.memset(sc0v[:, :, :, 0], 0.0)
        # mask_full[p, k, c] = 1 << k expanded over c
        nc.gpsimd.iota(iot[:], [[1, 12]], channel_multiplier=0)
        nc.gpsimd.memset(ones[:], 1)
        nc.vector.tensor_tensor(mask12[:], ones[:], iot[:], SHL)
        m12b = mask12[:].rearrange("p k -> p k ()").broadcast_to((P, 12, ccO))
        nc.gpsimd.tensor_copy(
            mask_full[:].rearrange("p (k c) -> p k c", c=ccO), m12b)

        # ---------- mega tiles ----------
        # per-load-chunk scan outputs (contiguous so combined scans can
        # flatten to the 2-D [partition, free] AP the scan op requires)
        so_t = {}
        for li, (cc, _, _) in enumerate(load_chunks):
            sot = pool.tile([P, 2 * cc * 24], f32, tag=f"so{li}")
            so_t[li] = sot[:].rearrange("p (i c b) -> p i c b", i=2, b=24)
        pt = pool.tile([P, C * 4], i32)             # limb cross products
        ptv = pt[:].rearrange("p (c i l) -> p c i l", i=2, l=2)
        ptT = pt[:].rearrange("p (c k) -> p k c", k=4)
        sh32 = pool.tile([P, 4 * C], i32)
        sh32v = sh32[:].rearrange("p (k c) -> p k c", k=4)
        G = pool.tile([P, 14 * C], i16)
        G16v = G[:].rearrange("p (s c) -> p s c", s=14)
        # G slots: 0..3 digits (d0=q0, d1, d2, d3), 4/8/12 = q1/q2/q3,
        # 7 t1a, 9 u2a, 10 u2, 11 c1, 13 c2

        def s_scan(sx, half):
            gc = load_chunks[sx][0]
            src = ab_t[sx][:, half].rearrange("p c b -> p (c b)")
            dst = so_t[sx][:, half].rearrange("p c b -> p (c b)")
            L(nc.vector.tensor_tensor_scan(
                dst, sc0[:][:, :gc * 24], src, 0.0, MUL, ADD),
              f"scan{'AB'[half]}{sx}")

        def s_scan_comb(sx):
            # one scan over [A-chunk | B-chunk]: the 0.0 multiplier at each
            # 12-bit limb start resets the recurrence, so the A->B seam
            # (a multiple of 12) resets automatically.
            gc = load_chunks[sx][0]
            src = ab_t[sx][:].rearrange("p i c b -> p (i c b)")
            dst = so_t[sx][:].rearrange("p i c b -> p (i c b)")
            L(nc.vector.tensor_tensor_scan(
                dst, sc0[:][:, :2 * gc * 24], src, 0.0, MUL, ADD),
              f"scanAB{sx}")

        def front_thunks(m):
            r0, r1 = moff[m], moff[m + 1]
            gc = r1 - r0
            v = nc.vector
            cols = slice(r0, r1)
            li = max(i for i in range(len(load_chunks)) if loff[i] <= r0)
            o0 = r0 - loff[li]
            lmA = so_t[li][:, 0, o0:o0 + gc, 11:24:12]   # [P, gc, 2]*2^-11
            lmB = so_t[li][:, 1, o0:o0 + gc, 11:24:12]
            ptg = ptv[:, r0:r1]
            ptTg = ptT[:, :, cols]
            shg = sh32v[:, :, cols]

            def prod(i):
                # (la*2^22)*lb == limb_a * limb_b exactly
                la_i = lmA[:, :, i:i + 1].broadcast_to((P, gc, 2))
                L(v.scalar_tensor_tensor(ptg[:, :, i], la_i,
                                         float(2 ** 22), lmB, MUL, MUL),
                  f"prod{m}_{i}")

            def sh():
                L(v.tensor_scalar(shg, ptTg, 12, None, SHR), f"sh{m}")

            def q():
                # q -> G slots (0,4,8,12): digit0 plus q1..q3
                L(v.scalar_tensor_tensor(G16v[:, 0:13:4, cols], shg,
                                         -4096.0, ptTg, MUL, ADD), f"q{m}")

            return [lambda: prod(0), lambda: prod(1), sh, q]

        def carry_thunks(m):
            r0, r1 = coff[m], coff[m + 1]
            v = nc.vector
            cols = slice(r0, r1)

            def gq(sn):
                return G16v[:, sn:sn + 1, cols]

            def sv(k):
                return sh32v[:, k:k + 1, cols]

            # carry lattice: d1 = q1+q2+sh0; c1 = d1>>12; d2 = u2a+q3+c1;
            # c2 = d2>>12; d3 = sh3+c2
            return [
                lambda: L(v.tensor_tensor(gq(7), gq(4), gq(8), ADD),
                          f"cy{m}_t1a"),
                lambda: L(v.tensor_tensor(gq(9), sv(1), sv(2), ADD),
                          f"cy{m}_u2a"),
                lambda: L(v.tensor_tensor(gq(10), gq(9), gq(12), ADD),
                          f"cy{m}_u2"),
                lambda: L(v.tensor_tensor(gq(1), gq(7), sv(0), ADD),
                          f"cy{m}_d1"),
                lambda: L(v.tensor_scalar(gq(11), gq(1), 12, None, SHR),
                          f"cy{m}_c1"),
                lambda: L(v.tensor_tensor(gq(2), gq(10), gq(11), ADD),
                          f"cy{m}_d2"),
                lambda: L(v.tensor_scalar(gq(13), gq(2), 12, None, SHR),
                          f"cy{m}_c2"),
                lambda: L(v.tensor_tensor(gq(3), sv(3), gq(13), ADD),
                          f"cy{m}_d3"),
            ]

        bt_t = {}
        ob_t = {}

        def s_band(sx):
            oc = out_sets[sx][0]
            o0, o1 = ooff[sx], ooff[sx + 1]
            eng = out_sets[sx][1]
            bt = pool.tile([P, 4 * 12 * oc], i16, tag=f"bt{sx}")
            bt4 = bt[:].rearrange("p (l k c) -> p l k c", l=4, k=12)
            bt_t[sx] = bt4
            digits = G16v[:, 0:4, o0:o1].rearrange("p l c -> p l () c")
            maskv = mask_full[:].rearrange(
                "p (k c) -> p () k c", c=ccO)[:, :, :, :oc]
            L(getattr(nc, eng).tensor_tensor(
                bt4, digits.broadcast_to((P, 4, 12, oc)),
                maskv.broadcast_to((P, 4, 12, oc)), AND), f"band{sx}")

        def s_sign(sx):
            eng = out_sets[sx][2]
            oc = out_sets[sx][0]
            ob = pool.tile([P, oc * 48], f32, tag=f"ob{sx}")
            ob_t[sx] = ob
            obT = ob[:].rearrange("p (c l k) -> p l k c", l=4, k=12)
            if eng == "scalar":
                L(nc.scalar.activation(obT, bt_t[sx], SIGN), f"sign{sx}")
            else:
                L(getattr(nc, eng).tensor_scalar(obT, bt_t[sx], 0, None,
                                                 NE), f"sign{sx}")

        def s_out(sx):
            o0, o1 = ooff[sx], ooff[sx + 1]
            eng = out_sets[sx][3]
            L(getattr(nc, eng).dma_start(
                Ov[:, o0:o1],
                ob_t[sx][:].rearrange("p (c b) -> p c b", b=48)),
              f"store{sx}")

        # ---------- emission ----------
        # The 8-op carry lattice is a serial chain with ~60-90ns write-ack
        # gaps; if big scans are schedulable during it, the static
        # scheduler slots them into every gap and stretches the chain by
        # microseconds.  The zipper interleaves the lattice with SMALL
        # ready ops (front prods/sh/q, band/sign/store triplets) and keeps
        # scans strictly between lattice sections.
        fronts_done = [False] * len(mid_batches)
        carries_done = [False] * len(carry_batches)
        carry_queued = [False] * len(carry_batches)
        outs_enabled = [False] * len(out_sets)
        chain = []      # [(m, thunk, is_last)]
        filler = []     # small-op thunks

        def front_cov():
            c = 0
            for i in range(len(mid_batches)):
                if not fronts_done[i]:
                    break
                c = moff[i + 1]
            return c

        def carry_cov():
            c = 0
            for i in range(len(carry_batches)):
                if not carries_done[i]:
                    break
                c = coff[i + 1]
            return c

        def on_emit():
            fc = front_cov()
            for m in range(len(carry_batches)):
                if not carry_queued[m] and coff[m + 1] <= fc:
                    ths = carry_thunks(m)
                    for i, t in enumerate(ths):
                        chain.append((m, t, i == len(ths) - 1))
                    carry_queued[m] = True
            cc = carry_cov()
            for o in range(len(out_sets)):
                if not outs_enabled[o] and ooff[o + 1] <= cc:
                    def trip(o=o):
                        s_band(o)
                        s_sign(o)
                        s_out(o)
                    filler.append(trip)
                    outs_enabled[o] = True

        def zip_emit():
            while chain or filler:
                if chain:
                    m, t, last = chain.pop(0)
                    t()
                    if last:
                        carries_done[m] = True
                    on_emit()
                if filler:
                    filler.pop(0)()
                    on_emit()

        if zip_emission:
            mid_next = 0
            for sx in range(len(scan_chunks)):
                if scan_chunks[sx][1] == "comb":
                    s_scan_comb(sx)
                else:
                    s_scan(sx, 0)
                    s_scan(sx, 1)
                while mid_next < len(mid_batches) and \
                        moff[mid_next + 1] <= soff[sx + 1]:
                    ths = front_thunks(mid_next)
                    mm = mid_next

                    def wrap_last(f, mm=mm):
                        def g():
                            f()
                            fronts_done[mm] = True
                            on_emit()
                        return g
                    ths[-1] = wrap_last(ths[-1])
                    filler.extend(ths)
                    mid_next += 1
                zip_emit()
            assert all(carries_done) and all(outs_enabled)
        else:
            mid_done = 0
            carry_done = 0
            out_done = 0
            for sx in range(len(scan_chunks)):
                if scan_chunks[sx][1] == "comb":
                    s_scan_comb(sx)
                else:
                    s_scan(sx, 0)
                    s_scan(sx, 1)
                ready = soff[sx + 1]
                while mid_done < len(mid_batches) and \
                        moff[mid_done + 1] <= ready:
                    for t in front_thunks(mid_done):
                        t()
                    mid_done += 1
                    while carry_done < len(carry_batches) and \
                            coff[carry_done + 1] <= moff[mid_done]:
                        for t in carry_thunks(carry_done):
                            t()
                        carry_done += 1
                        while out_done < len(out_sets) and \
                                ooff[out_done + 1] <= coff[carry_done]:
                            s_band(out_done)
                            s_sign(out_done)
                            s_out(out_done)
                            out_done += 1
            assert mid_done == len(mid_batches)
            assert out_done == len(out_sets)
            assert carry_done == len(carry_batches)

    nc.compile()
    return nc


_CACHE: dict = {}


def kernel(A: np.ndarray, B: np.ndarray) -> np.ndarray:
    A = np.ascontiguousarray(np.asarray(A, dtype=np.float32))
    B = np.ascontiguousarray(np.asarray(B, dtype=np.float32))
    assert A.shape == (BATCH, 24) and B.shape == (BATCH, 24), (A.shape, B.shape)

    if "nc" not in _CACHE:
        _CACHE["nc"] = build_nc(**DEFAULT_BUILD)
    nc = _CACHE["nc"]

    in_maps = []
    for i in range(N_CORES):
        sl = slice(i * NB, (i + 1) * NB)
        in_maps.append({"A": A[sl], "B": B[sl]})

    res = run_bass_kernel_spmd(nc, in_maps, core_ids=list(range(N_CORES)))
    outs = [np.asarray(res.results[i]["out"]) for i in range(N_CORES)]
    return np.concatenate(outs, axis=0).astype(np.float32)


if __name__ == "__main__":
    rng = np.random.default_rng(0)
    A = rng.integers(0, 2, (BATCH, 24)).astype(np.float32)
    B = rng.integers(0, 2, (BATCH, 24)).astype(np.float32)
    out = kernel(A, B)
    pw = (1 << np.arange(24)).astype(np.int64)
    a = (A.astype(np.int64) * pw).sum(-1)
    b = (B.astype(np.int64) * pw).sum(-1)
    p = a * b
    exp = ((p[:, None] >> np.arange(48)[None, :]) & 1).astype(np.float32)
    print("max abs diff:", np.abs(out - exp).max())
    assert np.array_equal(out, exp), "MISMATCH"
    print("EXACT MATCH")


# revision 3
# speedup vs baseline: 1.0331x; 1.0012x over previous
"""24x24-bit array multiplier on 8 TRN2 NeuronCores (Bass) — v2.

A, B: [65536, 24] float32 0/1 bit-vectors (LSB first) -> P: [65536, 48] bits.

Pure data parallel: each core gets 8192 rows as [128 part x 64 rows].
Math (exact): per row a = a0 + a1*2^12 (limbs via tensor_tensor_scan,
scaled 2^-11); 4 limb cross-products in f32 (exact, < 2^24) ->
radix-4096 digit lattice in i16 (every integer sum < 2^15: the DVE int
ADD runs through the fp32 datapath, so sums >= 2^24 would round) ->
per-digit 12-bit AND-mask expand (i16, 2x DVE mode) -> Sign (Act) or
not_equal (DVE) -> f32 bits.

Engine legality (neuronxcc): scans/shifts/bitwise/STT are DVE-only;
Act only does activation(func(scale*x+bias)); Pool does memset/iota/
converting-copies/f32-or-i32-matching arith TT and SWDGE DMA issue.

Schedule: loads tapered; first-chunk B load issued via Pool SWDGE so
both first transfers land early; all stage boundaries and queue
assignments are knobs; emission optionally zips the serial carry
lattice with small front/out ops so the static scheduler cannot
stretch it with 1.3us scan ops.
"""

import numpy as np

import concourse.bacc as bacc
import concourse.bass as bass
import concourse.mybir as mybir
import concourse.tile as tile
from concourse.alu_op_type import AluOpType
from concourse.bass_utils import run_bass_kernel_spmd

P = 128           # SBUF partitions
C = 64            # batch rows per partition
NB = P * C        # rows per core = 8192
N_CORES = 8
BATCH = NB * N_CORES

f32 = mybir.dt.float32
i32 = mybir.dt.int32
i16 = mybir.dt.int16

AND = AluOpType.bitwise_and
SHR = AluOpType.logical_shift_right
SHL = AluOpType.logical_shift_left
ADD, MUL = AluOpType.add, AluOpType.mult
NE = AluOpType.not_equal
SIGN = mybir.ActivationFunctionType.Sign

LABELS: dict = {}


def L(inst, label):
    try:
        LABELS[inst.ins.name] = label
    except AttributeError:
        pass
    return inst


DEFAULT_BUILD = dict(
    zip_emission=True,
    # (rows, engine_A, engine_B); engines in {sync, scalar, gpsimd}
    load_chunks=[(8, "sync", "gpsimd"), (24, "sync", "sync"),
                 (24, "sync", "sync"), (8, "sync", "sync")],
    # (rows, mode): aligned 1:1 with load_chunks; "split" = 2 scan ops
    # (A scan can start before B lands), "comb" = 1 op over [A|B]
    scan_chunks=[(8, "split", "-"), (24, "split", "-"),
                 (24, "split", "-"), (8, "split", "-")],
    # rows per front batch (prod/sh/q; must nest in a load chunk)
    mid_batches=[(8, "vector", "vector"), (24, "vector", "vector"),
                 (24, "vector", "vector"), (8, "vector", "vector")],
    # rows per carry batch (boundaries must be reachable from fronts)
    carry_batches=[32, 24, 4, 4],
    # (rows, band_eng, sign_eng, store_eng); sign in {scalar, vector}
    out_sets=[(8, "vector", "scalar", "sync"),
              (12, "vector", "scalar", "sync"),
              (12, "vector", "scalar", "sync"),
              (12, "vector", "scalar", "sync"),
              (12, "vector", "scalar", "sync"),
              (4, "vector", "vector", "sync"),
              (4, "vector", "vector", "scalar")],
)


def _offsets(entries):
    off = [0]
    for x in entries:
        off.append(off[-1] + (x[0] if isinstance(x, tuple) else x))
    return off


def build_nc(load_chunks=None, scan_chunks=None, mid_batches=None,
             out_sets=None, carry_batches=None,
             zip_emission=None) -> bass.Bass:
    """One SPMD program; every core runs it on its own shard."""
    d = DEFAULT_BUILD
    if zip_emission is None:
        zip_emission = d.get("zip_emission", False)
    load_chunks = load_chunks or d["load_chunks"]
    scan_chunks = scan_chunks or d["scan_chunks"]
    mid_batches = mid_batches or d["mid_batches"]
    out_sets = out_sets or d["out_sets"]
    carry_batches = carry_batches or d["carry_batches"]

    loff = _offsets(load_chunks)
    soff = _offsets(scan_chunks)
    moff = _offsets(mid_batches)
    ooff = _offsets(out_sets)
    coff = _offsets(carry_batches)
    assert loff[-1] == C and soff[-1] == C
    assert moff[-1] == C and ooff[-1] == C and coff[-1] == C
    # scan chunks align 1:1 with load chunks; mid batches nest in them
    assert [x[0] for x in scan_chunks] == [x[0] for x in load_chunks]
    for m in range(len(mid_batches)):
        li = max(i for i in range(len(load_chunks)) if loff[i] <= moff[m])
        assert moff[m + 1] <= loff[li + 1], (m, li, moff, loff)
    ccO = max(x[0] for x in out_sets)

    nc = bacc.Bacc(
        "TRN2",
        target_bir_lowering=False,
        debug=False,
        num_devices=N_CORES,
    )
    A = nc.declare_dram_parameter("A", [NB, 24], f32, isOutput=False)
    B = nc.declare_dram_parameter("B", [NB, 24], f32, isOutput=False)
    OUT = nc.declare_dram_parameter("out", [NB, 48], f32, isOutput=True)

    Av = A[:].rearrange("(p c) b -> p c b", p=P)
    Bv = B[:].rearrange("(p c) b -> p c b", p=P)
    Ov = OUT[:].rearrange("(p c) b -> p c b", p=P)

    with tile.TileContext(nc) as tc, \
            tc.tile_pool(name="const", bufs=1) as cpool, \
            tc.tile_pool(name="work", bufs=1) as pool:
        # ---------- load DMAs first (issue ASAP) ----------
        ab_t = {}
        for li, (cc, engA, engB) in enumerate(load_chunks):
            rows = slice(loff[li], loff[li + 1])
            ab = pool.tile([P, 2 * cc * 24], f32, tag=f"ab{li}")
            ab_t[li] = ab[:].rearrange("p (i c b) -> p i c b", i=2, b=24)
            L(getattr(nc, engA).dma_start(ab_t[li][:, 0], Av[:, rows]),
              f"loadA{li}")
            L(getattr(nc, engB).dma_start(ab_t[li][:, 1], Bv[:, rows]),
              f"loadB{li}")

        # ---------- constants ----------
        iot = cpool.tile([P, 12], i32)
        ones = cpool.tile([P, 12], i32)
        mask12 = cpool.tile([P, 12], i32)
        mask_full = cpool.tile([P, 12 * ccO], i16)
        scmax = max((2 if x[1] == "comb" else 1) * x[0]
                    for x in scan_chunks) * 24
        sc0 = cpool.tile([P, scmax], f32)

        # scan multiplier stream on Pool (DVE is the bottleneck engine):
        # 0.5 everywhere, 0.0 at each 12-bit limb start (resets recurrence)
        nc.gpsimd.memset(sc0[:], 0.5)
        sc0v = sc0[:].rearrange("p (c l b) -> p c l b", l=2, b=12)
        nc.gpsimd.memset(sc0v[:, :, :, 0], 0.0)
        # mask_full[p, k, c] = 1 << k expanded over c
        nc.gpsimd.iota(iot[:], [[1, 12]], channel_multiplier=0)
        nc.gpsimd.memset(ones[:], 1)
        nc.vector.tensor_tensor(mask12[:], ones[:], iot[:], SHL)
        m12b = mask12[:].rearrange("p k -> p k ()").broadcast_to((P, 12, ccO))
        nc.gpsimd.tensor_copy(
            mask_full[:].rearrange("p (k c) -> p k c", c=ccO), m12b)

        # ---------- mega tiles ----------
        # per-load-chunk scan outputs (contiguous so combined scans can
        # flatten to the 2-D [partition, free] AP the scan op requires)
        so_t = {}
        for li, (cc, _, _) in enumerate(load_chunks):
            sot = pool.tile([P, 2 * cc * 24], f32, tag=f"so{li}")
            so_t[li] = sot[:].rearrange("p (i c b) -> p i c b", i=2, b=24)
        pt = pool.tile([P, C * 4], i32)             # limb cross products
        ptv = pt[:].rearrange("p (c i l) -> p c i l", i=2, l=2)
        ptT = pt[:].rearrange("p (c k) -> p k c", k=4)
        sh32 = pool.tile([P, 4 * C], i32)
        sh32v = sh32[:].rearrange("p (k c) -> p k c", k=4)
        G = pool.tile([P, 14 * C], i16)
        G16v = G[:].rearrange("p (s c) -> p s c", s=14)
        # G slots: 0..3 digits (d0=q0, d1, d2, d3), 4/8/12 = q1/q2/q3,
        # 7 t1a, 9 u2a, 10 u2, 11 c1, 13 c2

        def s_scan(sx, half):
            gc = load_chunks[sx][0]
            src = ab_t[sx][:, half].rearrange("p c b -> p (c b)")
            dst = so_t[sx][:, half].rearrange("p c b -> p (c b)")
            L(nc.vector.tensor_tensor_scan(
                dst, sc0[:][:, :gc * 24], src, 0.0, MUL, ADD),
              f"scan{'AB'[half]}{sx}")

        def s_scan_comb(sx):
            # one scan over [A-chunk | B-chunk]: the 0.0 multiplier at each
            # 12-bit limb start resets the recurrence, so the A->B seam
            # (a multiple of 12) resets automatically.
            gc = load_chunks[sx][0]
            src = ab_t[sx][:].rearrange("p i c b -> p (i c b)")
            dst = so_t[sx][:].rearrange("p i c b -> p (i c b)")
            L(nc.vector.tensor_tensor_scan(
                dst, sc0[:][:, :2 * gc * 24], src, 0.0, MUL, ADD),
              f"scanAB{sx}")

        def front_thunks(m):
            r0, r1 = moff[m], moff[m + 1]
            gc = r1 - r0
            v = nc.vector
            cols = slice(r0, r1)
            li = max(i for i in range(len(load_chunks)) if loff[i] <= r0)
            o0 = r0 - loff[li]
            lmA = so_t[li][:, 0, o0:o0 + gc, 11:24:12]   # [P, gc, 2]*2^-11
            lmB = so_t[li][:, 1, o0:o0 + gc, 11:24:12]
            ptg = ptv[:, r0:r1]
            ptTg = ptT[:, :, cols]
            shg = sh32v[:, :, cols]

            def prod(i):
                # (la*2^22)*lb == limb_a * limb_b exactly
                la_i = lmA[:, :, i:i + 1].broadcast_to((P, gc, 2))
                L(v.scalar_tensor_tensor(ptg[:, :, i], la_i,
                                         float(2 ** 22), lmB, MUL, MUL),
                  f"prod{m}_{i}")

            def sh():
                L(v.tensor_scalar(shg, ptTg, 12, None, SHR), f"sh{m}")

            def q():
                # q -> G slots (0,4,8,12): digit0 plus q1..q3
                L(v.scalar_tensor_tensor(G16v[:, 0:13:4, cols], shg,
                                         -4096.0, ptTg, MUL, ADD), f"q{m}")

            return [lambda: prod(0), lambda: prod(1), sh, q]

        def carry_thunks(m):
            r0, r1 = coff[m], coff[m + 1]
            v = nc.vector
            cols = slice(r0, r1)

            def gq(sn):
                return G16v[:, sn:sn + 1, cols]

            def sv(k):
                return sh32v[:, k:k + 1, cols]

            # carry lattice: d1 = q1+q2+sh0; c1 = d1>>12; d2 = u2a+q3+c1;
            # c2 = d2>>12; d3 = sh3+c2
            return [
                lambda: L(v.tensor_tensor(gq(7), gq(4), gq(8), ADD),
                          f"cy{m}_t1a"),
                lambda: L(v.tensor_tensor(gq(9), sv(1), sv(2), ADD),
                          f"cy{m}_u2a"),
                lambda: L(v.tensor_tensor(gq(10), gq(9), gq(12), ADD),
                          f"cy{m}_u2"),
                lambda: L(v.tensor_tensor(gq(1), gq(7), sv(0), ADD),
                          f"cy{m}_d1"),
                lambda: L(v.tensor_scalar(gq(11), gq(1), 12, None, SHR),
                          f"cy{m}_c1"),
                lambda: L(v.tensor_tensor(gq(2), gq(10), gq(11), ADD),
                          f"cy{m}_d2"),
                lambda: L(v.tensor_scalar(gq(13), gq(2), 12, None, SHR),
                          f"cy{m}_c2"),
                lambda: L(v.tensor_tensor(gq(3), sv(3), gq(13), ADD),
                          f"cy{m}_d3"),
            ]

        bt_t = {}
        ob_t = {}

        def s_band(sx):
            oc = out_sets[sx][0]
            o0, o1 = ooff[sx], ooff[sx + 1]
            eng = out_sets[sx][1]
            bt = pool.tile([P, 4 * 12 * oc], i16, tag=f"bt{sx}")
            bt4 = bt[:].rearrange("p (l k c) -> p l k c", l=4, k=12)
            bt_t[sx] = bt4
            digits = G16v[:, 0:4, o0:o1].rearrange("p l c -> p l () c")
            maskv = mask_full[:].rearrange(
                "p (k c) -> p () k c", c=ccO)[:, :, :, :oc]
            L(getattr(nc, eng).tensor_tensor(
                bt4, digits.broadcast_to((P, 4, 12, oc)),
                maskv.broadcast_to((P, 4, 12, oc)), AND), f"band{sx}")

        def s_sign(sx):
            eng = out_sets[sx][2]
            oc = out_sets[sx][0]
            ob = pool.tile([P, oc * 48], f32, tag=f"ob{sx}")
            ob_t[sx] = ob
            obT = ob[:].rearrange("p (c l k) -> p l k c", l=4, k=12)
            if eng == "scalar":
                L(nc.scalar.activation(obT, bt_t[sx], SIGN), f"sign{sx}")
            else:
                L(getattr(nc, eng).tensor_scalar(obT, bt_t[sx], 0, None,
                                                 NE), f"sign{sx}")

        def s_out(sx):
            o0, o1 = ooff[sx], ooff[sx + 1]
            eng = out_sets[sx][3]
            L(getattr(nc, eng).dma_start(
                Ov[:, o0:o1],
                ob_t[sx][:].rearrange("p (c b) -> p c b", b=48)),
              f"store{sx}")

        # ---------- emission ----------
        # The 8-op carry lattice is a serial chain with ~60-90ns write-ack
        # gaps; if big scans are schedulable during it, the static
        # scheduler slots them into every gap and stretches the chain by
        # microseconds.  The zipper interleaves the lattice with SMALL
        # ready ops (front prods/sh/q, band/sign/store triplets) and keeps
        # scans strictly between lattice sections.
        fronts_done = [False] * len(mid_batches)
        carries_done = [False] * len(carry_batches)
        carry_queued = [False] * len(carry_batches)
        outs_enabled = [False] * len(out_sets)
        chain = []      # [(m, thunk, is_last)]
        filler = []     # small-op thunks

        def front_cov():
            c = 0
            for i in range(len(mid_batches)):
                if not fronts_done[i]:
                    break
                c = moff[i + 1]
            return c

        def carry_cov():
            c = 0
            for i in range(len(carry_batches)):
                if not carries_done[i]:
                    break
                c = coff[i + 1]
            return c

        def on_emit():
            fc = front_cov()
            newly = []
            for m in range(len(carry_batches)):
                if not carry_queued[m] and coff[m + 1] <= fc:
                    newly.append((m, carry_thunks(m)))
                    carry_queued[m] = True
            if newly:
                # interleave simultaneously-enabled lattices round-robin so
                # each fills the other's dependency-ack bubbles (keeps the
                # greedy static scheduler from slotting 1.3us scans there)
                n = max(len(t) for _, t in newly)
                for i in range(n):
                    for m, ths in newly:
                        if i < len(ths):
                            chain.append((m, ths[i], i == len(ths) - 1))
            cc = carry_cov()
            for o in range(len(out_sets)):
                if not outs_enabled[o] and ooff[o + 1] <= cc:
                    def trip(o=o):
                        s_band(o)
                        s_sign(o)
                        s_out(o)
                    filler.append(trip)
                    outs_enabled[o] = True

        def zip_emit():
            while chain or filler:
                if chain:
                    m, t, last = chain.pop(0)
                    t()
                    if last:
                        carries_done[m] = True
                    on_emit()
                if filler:
                    filler.pop(0)()
                    on_emit()

        if zip_emission:
            mid_next = 0
            for sx in range(len(scan_chunks)):
                if scan_chunks[sx][1] == "comb":
                    s_scan_comb(sx)
                else:
                    s_scan(sx, 0)
                    s_scan(sx, 1)
                while mid_next < len(mid_batches) and \
                        moff[mid_next + 1] <= soff[sx + 1]:
                    ths = front_thunks(mid_next)
                    mm = mid_next

                    def wrap_last(f, mm=mm):
                        def g():
                            f()
                            fronts_done[mm] = True
                            on_emit()
                        return g
                    ths[-1] = wrap_last(ths[-1])
                    filler.extend(ths)
                    mid_next += 1
                zip_emit()
            assert all(carries_done) and all(outs_enabled)
        else:
            mid_done = 0
            carry_done = 0
            out_done = 0
            for sx in range(len(scan_chunks)):
                if scan_chunks[sx][1] == "comb":
                    s_scan_comb(sx)
                else:
                    s_scan(sx, 0)
                    s_scan(sx, 1)
                ready = soff[sx + 1]
                while mid_done < len(mid_batches) and \
                        moff[mid_done + 1] <= ready:
                    for t in front_thunks(mid_done):
                        t()
                    mid_done += 1
                    while carry_done < len(carry_batches) and \
                            coff[carry_done + 1] <= moff[mid_done]:
                        for t in carry_thunks(carry_done):
                            t()
                        carry_done += 1
                        while out_done < len(out_sets) and \
                                ooff[out_done + 1] <= coff[carry_done]:
                            s_band(out_done)
                            s_sign(out_done)
                            s_out(out_done)
                            out_done += 1
            assert mid_done == len(mid_batches)
            assert out_done == len(out_sets)
            assert carry_done == len(carry_batches)

    nc.compile()
    return nc


_CACHE: dict = {}


def kernel(A: np.ndarray, B: np.ndarray) -> np.ndarray:
    A = np.ascontiguousarray(np.asarray(A, dtype=np.float32))
    B = np.ascontiguousarray(np.asarray(B, dtype=np.float32))
    assert A.shape == (BATCH, 24) and B.shape == (BATCH, 24), (A.shape, B.shape)

    if "nc" not in _CACHE:
        _CACHE["nc"] = build_nc(**DEFAULT_BUILD)
    nc = _CACHE["nc"]

    in_maps = []
    for i in range(N_CORES):
        sl = slice(i * NB, (i + 1) * NB)
        in_maps.append({"A": A[sl], "B": B[sl]})

    res = run_bass_kernel_spmd(nc, in_maps, core_ids=list(range(N_CORES)))
    outs = [np.asarray(res.results[i]["out"]) for i in range(N_CORES)]
    return np.concatenate(outs, axis=0).astype(np.float32)


if __name__ == "__main__":
    rng = np.random.default_rng(0)
    A = rng.integers(0, 2, (BATCH, 24)).astype(np.float32)
    B = rng.integers(0, 2, (BATCH, 24)).astype(np.float32)
    out = kernel(A, B)
    pw = (1 << np.arange(24)).astype(np.int64)
    a = (A.astype(np.int64) * pw).sum(-1)
    b = (B.astype(np.int64) * pw).sum(-1)
    p = a * b
    exp = ((p[:, None] >> np.arange(48)[None, :]) & 1).astype(np.float32)
    print("max abs diff:", np.abs(out - exp).max())
    assert np.array_equal(out, exp), "MISMATCH"
    print("EXACT MATCH")
